# revision 12
# baseline (speedup 1.0000x reference)
"""CMADE ensemble kernel for 8 TRN2 NeuronCores.

Problem: B=16 binary-masked 4-layer MLPs (96 -> 1024 -> 1024 -> 1024 -> 64)
over the same N=4096 batch; output = mean over the 16 masks.

Strategy: shard the mask/ensemble dim B across the 8 cores (2 masks/core).
Each core computes its two masked MLPs entirely on-chip in bf16 (fp32 PSUM
accumulation), sums the two final-layer outputs, scales by 1/16, and an
AllReduce(add) over the 8 cores produces the ensemble mean on every core.

Layout: activations are kept transposed ([feature, batch]) so weights are the
stationary matmul operand; masked weights (W.T * M) are produced on-chip by
the vector engine from resident W.T tiles and streamed mask tiles.
"""

import numpy as np
import ml_dtypes

from concourse import bacc
import concourse.bass as bass
import concourse.mybir as mybir
import concourse.tile as tile
from concourse.bass_utils import run_bass_kernel_spmd
from concourse.masks import make_identity

BF16 = ml_dtypes.bfloat16

N = 4096
B = 16
NCORES = 8
B_LOC = B // NCORES          # 2 masks per core
D_IN = 96
H = 1024
D_OUT = 64
KT = H // 128                # 8 k-tiles for the 1024-wide dims
NCHUNK = 512                 # batch columns per chunk
NCH = N // NCHUNK            # 8 chunks

# set True (or env BASS_TRACE=1) before calling kernel() to capture an NTFF
# profile; exec time lands in LAST_RESULT.exec_time_ns
TRACE = False
LAST_RESULT = None

_CACHE = {}


def _ensure_ntff_hook():
    """The agent image's antenv lacks axon_hooks; reconstruct the NTFF
    profile hook from trn_agent_boot so trace=True yields exec_time_ns."""
    import sys as _sys
    import types
    try:
        from antenv import axon_hooks  # noqa: F401
        return
    except ImportError:
        pass
    import antenv
    import concourse.bass_utils as _bu
    _bu.upload_artifacts = lambda tmpdir: tmpdir  # zero-egress container
    holder = {}
    mod = types.ModuleType("antenv.axon_hooks")
    mod.set_axon_ntff_profile_hook = lambda h: holder.__setitem__("h", h)
    mod.get_axon_ntff_profile_hook = lambda: holder.get("h")
    _sys.modules["antenv.axon_hooks"] = mod
    antenv.axon_hooks = mod
    from trn_agent_boot.trn_boot import _ntff_profile_via_ctypes
    mod.set_axon_ntff_profile_hook(
        _ntff_profile_via_ctypes("/opt/axon/libaxon_pjrt.so"))


def _build_graph():
    f32 = mybir.dt.float32
    bf = mybir.dt.bfloat16
    nc = bacc.Bacc("TRN2", target_bir_lowering=False, debug=False,
                   num_devices=NCORES)

    # ---- I/O ----
    xyT_d = nc.dram_tensor("xyT", [D_IN, N], bf, kind="ExternalInput")
    wt0_d = nc.dram_tensor("wt0", [D_IN, H], bf, kind="ExternalInput")
    wt1_d = nc.dram_tensor("wt1", [128, KT * H], bf, kind="ExternalInput")
    wt2_d = nc.dram_tensor("wt2", [128, KT * H], bf, kind="ExternalInput")
    wt3_d = nc.dram_tensor("wt3", [128, KT * D_OUT], bf, kind="ExternalInput")
    m0_d = nc.dram_tensor("m0", [D_IN, B_LOC * H], bf, kind="ExternalInput")
    m1_d = nc.dram_tensor("m1", [128, B_LOC * KT * H], bf, kind="ExternalInput")
    m2_d = nc.dram_tensor("m2", [128, B_LOC * KT * H], bf, kind="ExternalInput")
    m3_d = nc.dram_tensor("m3", [128, B_LOC * KT * D_OUT], bf, kind="ExternalInput")
    b0_d = nc.dram_tensor("b0r", [128, KT], f32, kind="ExternalInput")
    b1_d = nc.dram_tensor("b1r", [128, KT], f32, kind="ExternalInput")
    b2_d = nc.dram_tensor("b2r", [128, KT], f32, kind="ExternalInput")
    b3_d = nc.dram_tensor("b3r", [D_OUT, 1], f32, kind="ExternalInput")
    # each core returns its ReduceScatter shard: rows [c*512 + core*64, +64)
    out_d = nc.dram_tensor("out", [NCH, D_OUT, D_OUT], f32,
                           kind="ExternalOutput")

    relu = mybir.ActivationFunctionType.Relu
    copyf = mybir.ActivationFunctionType.Identity

    from contextlib import ExitStack
    with tile.TileContext(nc) as tc, ExitStack() as ctx:
        const = ctx.enter_context(tc.tile_pool(name="const", bufs=1))
        mwp = ctx.enter_context(tc.tile_pool(name="mw", bufs=1))
        mstg = ctx.enter_context(tc.tile_pool(name="mstg", bufs=3))
        apool = ctx.enter_context(tc.tile_pool(name="act", bufs=2))
        pspool = ctx.enter_context(tc.tile_pool(name="ps", bufs=4, space="PSUM"))
        ps3pool = ctx.enter_context(tc.tile_pool(name="ps3", bufs=2, space="PSUM"))
        pstp = ctx.enter_context(tc.tile_pool(name="pst", bufs=2, space="PSUM"))
        finp = ctx.enter_context(tc.tile_pool(name="fin", bufs=2))
        outtp = ctx.enter_context(tc.tile_pool(name="outT", bufs=4))
        dram = ctx.enter_context(tc.tile_pool(name="dram", bufs=1, space="DRAM"))

        # per-chunk bounce tensors: a single tensor would create
        # tensor-granularity WAR deps between chunk c's collective and
        # chunk c+1's staging DMAs, serializing the pipeline
        red_in = [dram.tile([NCHUNK, D_OUT], f32, tag=f"red_in{c}",
                            name=f"red_in{c}") for c in range(NCH)]
        red_out = [dram.tile([D_OUT, D_OUT], f32, tag=f"red_out{c}",
                             name=f"red_out{c}") for c in range(NCH)]

        # ---- resident constants + masked weights ----
        # DMA emission order matters: the critical path for chunk 0 / mask 0
        # is xyT, wt0+m0(b0), then wt1+m1(b0); defer wt2/wt3 and all b=1
        # masks. Weights go on the sync queue, masks on the gpsimd queue so
        # the big weight transfers don't delay mask staging.
        mw0 = mwp.tile([D_IN, B_LOC * H], bf, tag="mw0")
        mw1 = mwp.tile([128, B_LOC * KT * H], bf, tag="mw1")
        mw2 = mwp.tile([128, B_LOC * KT * H], bf, tag="mw2")
        mw3 = mwp.tile([128, B_LOC * KT * D_OUT], bf, tag="mw3")

        xyT = const.tile([D_IN, N], bf, tag="xyT")
        nc.sync.dma_start(xyT[:], xyT_d[:])
        wt0 = const.tile([D_IN, H], bf, tag="wt0")
        nc.sync.dma_start(wt0[:], wt0_d[:])
        b0t = const.tile([128, KT], f32, tag="b0t")
        nc.sync.dma_start(b0t[:], b0_d[:])
        b1t = const.tile([128, KT], f32, tag="b1t")
        nc.sync.dma_start(b1t[:], b1_d[:])
        b2t = const.tile([128, KT], f32, tag="b2t")
        nc.sync.dma_start(b2t[:], b2_d[:])
        b3t = const.tile([D_OUT, 1], f32, tag="b3t")
        nc.sync.dma_start(b3t[:], b3_d[:])

        def mul0(b):
            mt = mstg.tile([D_IN, H], bf, tag="m0s")
            nc.gpsimd.dma_start(mt[:], m0_d[:, b * H:(b + 1) * H])
            nc.vector.tensor_mul(mw0[:, b * H:(b + 1) * H], wt0[:], mt[:])

        def mul12(b, k, wt, m_d, mw, tagn):
            mt = mstg.tile([128, H], bf, tag=tagn)
            lo = (b * KT + k) * H
            nc.gpsimd.dma_start(mt[:], m_d[:, lo:lo + H])
            nc.vector.tensor_mul(mw[:, lo:lo + H],
                                 wt[:, k * H:(k + 1) * H], mt[:])

        def mul3(b):
            mt = mstg.tile([128, KT * D_OUT], bf, tag="m3s")
            lo = b * KT * D_OUT
            nc.gpsimd.dma_start(mt[:], m3_d[:, lo:lo + KT * D_OUT])
            nc.vector.tensor_mul(mw3[:, lo:lo + KT * D_OUT], wt3[:], mt[:])

        mul0(0)
        wt1 = const.tile([128, KT * H], bf, tag="wt1")
        nc.sync.dma_start(wt1[:], wt1_d[:])
        mul0(1)
        for k in range(KT):
            mul12(0, k, wt1, m1_d, mw1, "m1s")
        wt2 = const.tile([128, KT * H], bf, tag="wt2")
        nc.sync.dma_start(wt2[:], wt2_d[:])
        for k in range(KT):
            mul12(0, k, wt2, m2_d, mw2, "m2s")
        wt3 = const.tile([128, KT * D_OUT], bf, tag="wt3")
        nc.sync.dma_start(wt3[:], wt3_d[:])
        mul3(0)
        for k in range(KT):
            mul12(1, k, wt1, m1_d, mw1, "m1s")
        for k in range(KT):
            mul12(1, k, wt2, m2_d, mw2, "m2s")
        mul3(1)

        ident = const.tile([128, 128], f32, tag="ident")
        make_identity(nc, ident[:])

        # ---- main compute ----
        for c in range(NCH):
            cs = bass.ts(c, NCHUNK)
            ps3 = ps3pool.tile([D_OUT, NCHUNK], f32, tag="ps3")
            for b in range(B_LOC):
                # layer 0: [96] -> [1024]
                a0 = []
                for m in range(KT):
                    ps = pspool.tile([128, NCHUNK], f32, tag="ps")
                    nc.tensor.matmul(
                        ps[:],
                        mw0[:, b * H + m * 128: b * H + (m + 1) * 128],
                        xyT[:, cs], start=True, stop=True)
                    at = apool.tile([128, NCHUNK], bf, tag=f"a0_{m}")
                    nc.scalar.activation(at[:], ps[:], relu,
                                         bias=b0t[:, m:m + 1])
                    a0.append(at)
                # layer 1: [1024] -> [1024]
                a1 = []
                for m in range(KT):
                    ps = pspool.tile([128, NCHUNK], f32, tag="ps")
                    for k in range(KT):
                        lo = (b * KT + k) * H + m * 128
                        nc.tensor.matmul(ps[:], mw1[:, lo:lo + 128], a0[k][:],
                                         start=(k == 0), stop=(k == KT - 1))
                    at = apool.tile([128, NCHUNK], bf, tag=f"a1_{m}")
                    nc.scalar.activation(at[:], ps[:], relu,
                                         bias=b1t[:, m:m + 1])
                    a1.append(at)
                # layer 2: [1024] -> [1024]
                a2 = []
                for m in range(KT):
                    ps = pspool.tile([128, NCHUNK], f32, tag="ps")
                    for k in range(KT):
                        lo = (b * KT + k) * H + m * 128
                        nc.tensor.matmul(ps[:], mw2[:, lo:lo + 128], a1[k][:],
                                         start=(k == 0), stop=(k == KT - 1))
                    at = apool.tile([128, NCHUNK], bf, tag=f"a2_{m}")
                    nc.scalar.activation(at[:], ps[:], relu,
                                         bias=b2t[:, m:m + 1])
                    a2.append(at)
                # layer 3: [1024] -> [64]; accumulate both masks in one psum
                for k in range(KT):
                    lo = (b * KT + k) * D_OUT
                    nc.tensor.matmul(ps3[:], mw3[:, lo:lo + D_OUT], a2[k][:],
                                     start=(b == 0 and k == 0),
                                     stop=(b == B_LOC - 1 and k == KT - 1))
            # finalize chunk: scale by 1/16, add b3/8 (summed to b3 over cores)
            fin = finp.tile([D_OUT, NCHUNK], f32, tag="fin")
            nc.scalar.activation(fin[:], ps3[:], copyf, bias=b3t[:, 0:1],
                                 scale=1.0 / B)
            # transpose [64, 512] -> 4x [128, 64] and stage to DRAM
            for t in range(NCHUNK // 128):
                pst = pstp.tile([128, D_OUT], f32, tag="pst")
                nc.tensor.transpose(pst[:], fin[:, t * 128:(t + 1) * 128],
                                    ident[:D_OUT, :D_OUT])
                ot = outtp.tile([128, D_OUT], f32, tag="ot")
                nc.vector.tensor_copy(ot[:], pst[:])
                nc.sync.dma_start(red_in[c][t * 128:(t + 1) * 128, :], ot[:])
            # pipelined ensemble-sum: each core receives rows
            # [c*512 + core*64, +64) of the chunk, overlapped with the
            # next chunk's compute
            nc.gpsimd.collective_compute(
                "ReduceScatter", mybir.AluOpType.add,
                replica_groups=[list(range(NCORES))],
                ins=[red_in[c][:].opt()], outs=[red_out[c][:].opt()])
            nc.sync.dma_start(out_d[c], red_out[c][:])

    nc.compile()
    return nc


def _prep_shared(xy, W0, W1, W2, W3, b0, b1, b2, b3):
    xyT = np.ascontiguousarray(xy.T).astype(BF16)
    wt0 = np.ascontiguousarray(W0.T).astype(BF16)

    def kfold(wT, out_w):
        # [1024, out] -> [8, 128, out] -> [128, 8*out]
        return np.ascontiguousarray(
            wT.reshape(KT, 128, out_w).transpose(1, 0, 2).reshape(128, KT * out_w)
        ).astype(BF16)

    wt1 = kfold(W1.T, H)
    wt2 = kfold(W2.T, H)
    wt3 = kfold(W3.T, D_OUT)
    b0r = np.ascontiguousarray(b0.reshape(KT, 128).T).astype(np.float32)
    b1r = np.ascontiguousarray(b1.reshape(KT, 128).T).astype(np.float32)
    b2r = np.ascontiguousarray(b2.reshape(KT, 128).T).astype(np.float32)
    b3r = np.ascontiguousarray((b3 / NCORES).reshape(D_OUT, 1)).astype(np.float32)
    return dict(xyT=xyT, wt0=wt0, wt1=wt1, wt2=wt2, wt3=wt3,
                b0r=b0r, b1r=b1r, b2r=b2r, b3r=b3r)


def _prep_masks(mask0, mask1, mask2, mask3, core):
    s = slice(core * B_LOC, (core + 1) * B_LOC)

    def mkfold(m, out_w):
        # [b, 1024, out] -> [b, 8, 128, out] -> [128, b*8*out]
        return np.ascontiguousarray(
            m.reshape(B_LOC, KT, 128, out_w).transpose(2, 0, 1, 3)
            .reshape(128, B_LOC * KT * out_w)).astype(BF16)

    m0 = np.ascontiguousarray(
        mask0[s].transpose(1, 0, 2).reshape(D_IN, B_LOC * H)).astype(BF16)
    m1 = mkfold(mask1[s], H)
    m2 = mkfold(mask2[s], H)
    m3 = mkfold(mask3[s], D_OUT)
    return dict(m0=m0, m1=m1, m2=m2, m3=m3)


def kernel(xy, W0, b0, W1, b1, W2, b2, W3, b3,
           mask0, mask1, mask2, mask3):
    global LAST_RESULT
    xy = np.asarray(xy, np.float32)
    args = [np.asarray(a, np.float32) for a in
            (W0, W1, W2, W3, b0, b1, b2, b3)]
    masks = [np.asarray(m, np.float32) for m in (mask0, mask1, mask2, mask3)]

    if "nc" not in _CACHE:
        _CACHE["nc"] = _build_graph()
    nc = _CACHE["nc"]

    shared = _prep_shared(xy, *args)
    in_maps = []
    for core in range(NCORES):
        im = dict(shared)
        im.update(_prep_masks(*masks, core))
        in_maps.append(im)

    if TRACE:
        _ensure_ntff_hook()
    res = run_bass_kernel_spmd(
        nc, in_maps, core_ids=list(range(NCORES)),
        trace=TRACE)
    LAST_RESULT = res
    # reassemble: core i's out[c] holds rows [c*512 + i*64, +64)
    parts = np.stack([np.asarray(res.results[i]["out"], np.float32)
                      for i in range(NCORES)])       # [core, chunk, 64, 64]
    return np.ascontiguousarray(
        parts.transpose(1, 0, 2, 3).reshape(N, D_OUT))


# revision 13
# speedup vs baseline: 1.1971x; 1.1971x over previous
"""CMADE ensemble kernel for 8 TRN2 NeuronCores.

Problem: B=16 binary-masked 4-layer MLPs (96 -> 1024 -> 1024 -> 1024 -> 64)
over the same N=4096 batch; output = mean over the 16 masks.

Strategy: data-parallel over the batch N — each core takes 512 rows and runs
all 16 masked MLPs on them, accumulating the final-layer outputs of all 16
masks into a single PSUM tile. The ensemble mean is finished locally
(scale 1/16 + bias), so no inter-core collective is needed; the host
concatenates the 8 row-slices. Masked weights (W.T * M, bf16) are produced
on-chip by the vector engine from resident W.T tiles and streamed mask
tiles, double-buffered across masks. Matmuls run in bf16 with fp32 PSUM
accumulation; activations stay transposed ([feature, batch]) so weights are
the stationary operand.
"""

import numpy as np
import ml_dtypes

from concourse import bacc
import concourse.bass as bass
import concourse.mybir as mybir
import concourse.tile as tile
from concourse.bass_utils import run_bass_kernel_spmd
from concourse.masks import make_identity

BF16 = ml_dtypes.bfloat16

N = 4096
B = 16
NCORES = 8
NLOC = N // NCORES           # 512 batch rows per core
D_IN = 96
H = 1024
D_OUT = 64
KT = H // 128                # 8 k-tiles for the 1024-wide dims

# set True (or env BASS_TRACE=1) before calling kernel() to capture an NTFF
# profile; exec time lands in LAST_RESULT.exec_time_ns
TRACE = False
LAST_RESULT = None

_CACHE = {}


def _ensure_ntff_hook():
    """The agent image's antenv lacks axon_hooks; reconstruct the NTFF
    profile hook from trn_agent_boot so trace=True yields exec_time_ns."""
    import sys as _sys
    import types
    try:
        from antenv import axon_hooks  # noqa: F401
        return
    except ImportError:
        pass
    import antenv
    import concourse.bass_utils as _bu
    _bu.upload_artifacts = lambda tmpdir: tmpdir  # zero-egress container
    holder = {}
    mod = types.ModuleType("antenv.axon_hooks")
    mod.set_axon_ntff_profile_hook = lambda h: holder.__setitem__("h", h)
    mod.get_axon_ntff_profile_hook = lambda: holder.get("h")
    _sys.modules["antenv.axon_hooks"] = mod
    antenv.axon_hooks = mod
    from trn_agent_boot.trn_boot import _ntff_profile_via_ctypes
    mod.set_axon_ntff_profile_hook(
        _ntff_profile_via_ctypes("/opt/axon/libaxon_pjrt.so"))


def _build_graph():
    f32 = mybir.dt.float32
    bf = mybir.dt.bfloat16
    nc = bacc.Bacc("TRN2", target_bir_lowering=False, debug=False,
                   num_devices=NCORES)

    # ---- I/O ----
    xyT_d = nc.dram_tensor("xyT", [D_IN, NLOC], bf, kind="ExternalInput")
    wt0_d = nc.dram_tensor("wt0", [D_IN, H], bf, kind="ExternalInput")
    wt1_d = nc.dram_tensor("wt1", [128, KT * H], bf, kind="ExternalInput")
    wt2_d = nc.dram_tensor("wt2", [128, KT * H], bf, kind="ExternalInput")
    wt3_d = nc.dram_tensor("wt3", [128, KT * D_OUT], bf, kind="ExternalInput")
    m0_d = nc.dram_tensor("m0", [B, D_IN, H], bf, kind="ExternalInput")
    m1_d = nc.dram_tensor("m1", [B, 128, KT * H], bf, kind="ExternalInput")
    m2_d = nc.dram_tensor("m2", [B, 128, KT * H], bf, kind="ExternalInput")
    m3_d = nc.dram_tensor("m3", [B, 128, KT * D_OUT], bf, kind="ExternalInput")
    b0_d = nc.dram_tensor("b0r", [128, KT], f32, kind="ExternalInput")
    b1_d = nc.dram_tensor("b1r", [128, KT], f32, kind="ExternalInput")
    b2_d = nc.dram_tensor("b2r", [128, KT], f32, kind="ExternalInput")
    b3_d = nc.dram_tensor("b3r", [D_OUT, 1], f32, kind="ExternalInput")
    out_d = nc.dram_tensor("out", [NLOC, D_OUT], f32, kind="ExternalOutput")

    relu = mybir.ActivationFunctionType.Relu
    iden = mybir.ActivationFunctionType.Identity

    from contextlib import ExitStack
    with tile.TileContext(nc) as tc, ExitStack() as ctx:
        const = ctx.enter_context(tc.tile_pool(name="const", bufs=1))
        mwp = ctx.enter_context(tc.tile_pool(name="mw", bufs=2))
        mstg = ctx.enter_context(tc.tile_pool(name="mstg", bufs=4))
        apool = ctx.enter_context(tc.tile_pool(name="act", bufs=2))
        pspool = ctx.enter_context(tc.tile_pool(name="ps", bufs=4, space="PSUM"))
        ps3pool = ctx.enter_context(tc.tile_pool(name="ps3", bufs=1, space="PSUM"))
        pstp = ctx.enter_context(tc.tile_pool(name="pst", bufs=2, space="PSUM"))
        finp = ctx.enter_context(tc.tile_pool(name="fin", bufs=1))
        outtp = ctx.enter_context(tc.tile_pool(name="outT", bufs=4))

        # ---- resident constants ----
        xyT = const.tile([D_IN, NLOC], bf, tag="xyT")
        nc.sync.dma_start(xyT[:], xyT_d[:])
        wt0 = const.tile([D_IN, H], bf, tag="wt0")
        nc.sync.dma_start(wt0[:], wt0_d[:])
        b0t = const.tile([128, KT], f32, tag="b0t")
        nc.sync.dma_start(b0t[:], b0_d[:])
        b1t = const.tile([128, KT], f32, tag="b1t")
        nc.sync.dma_start(b1t[:], b1_d[:])
        b2t = const.tile([128, KT], f32, tag="b2t")
        nc.sync.dma_start(b2t[:], b2_d[:])
        b3t = const.tile([D_OUT, 1], f32, tag="b3t")
        nc.sync.dma_start(b3t[:], b3_d[:])
        wt1 = const.tile([128, KT * H], bf, tag="wt1")
        nc.sync.dma_start(wt1[:], wt1_d[:])
        wt2 = const.tile([128, KT * H], bf, tag="wt2")
        nc.gpsimd.dma_start(wt2[:], wt2_d[:])
        wt3 = const.tile([128, KT * D_OUT], bf, tag="wt3")
        nc.gpsimd.dma_start(wt3[:], wt3_d[:])
        ident = const.tile([128, 128], f32, tag="ident")
        make_identity(nc, ident[:])

        # masked-weight producer for mask b (bf16, vector engine); masks
        # stream on the sync/gpsimd DMA queues, double-buffered via pools
        def make_mw(b):
            mw0 = mwp.tile([D_IN, H], bf, tag="mw0", name=f"mw0_{b}")
            mt0 = mstg.tile([D_IN, H], bf, tag="m0s", name=f"m0s_{b}")
            nc.sync.dma_start(mt0[:], m0_d[b])
            nc.vector.tensor_mul(mw0[:], wt0[:], mt0[:])
            mw1 = mwp.tile([128, KT * H], bf, tag="mw1", name=f"mw1_{b}")
            mw2 = mwp.tile([128, KT * H], bf, tag="mw2", name=f"mw2_{b}")
            for k in range(KT):
                mt = mstg.tile([128, H], bf, tag="m1s", name=f"m1s_{b}_{k}")
                nc.sync.dma_start(mt[:], m1_d[b][:, k * H:(k + 1) * H])
                nc.vector.tensor_mul(mw1[:, k * H:(k + 1) * H],
                                     wt1[:, k * H:(k + 1) * H], mt[:])
                mt = mstg.tile([128, H], bf, tag="m2s", name=f"m2s_{b}_{k}")
                nc.gpsimd.dma_start(mt[:], m2_d[b][:, k * H:(k + 1) * H])
                nc.vector.tensor_mul(mw2[:, k * H:(k + 1) * H],
                                     wt2[:, k * H:(k + 1) * H], mt[:])
            mw3 = mwp.tile([128, KT * D_OUT], bf, tag="mw3", name=f"mw3_{b}")
            mt3 = mstg.tile([128, KT * D_OUT], bf, tag="m3s", name=f"m3s_{b}")
            nc.gpsimd.dma_start(mt3[:], m3_d[b])
            nc.vector.tensor_mul(mw3[:], wt3[:], mt3[:])
            return mw0, mw1, mw2, mw3

        # ---- main compute: all 16 masks over this core's 512 rows ----
        ps3 = ps3pool.tile([D_OUT, NLOC], f32, tag="ps3")
        for b in range(B):
            mw0, mw1, mw2, mw3 = make_mw(b)
            # layer 0: [96] -> [1024]
            a0 = []
            for m in range(KT):
                ps = pspool.tile([128, NLOC], f32, tag="ps")
                nc.tensor.matmul(ps[:], mw0[:, m * 128:(m + 1) * 128],
                                 xyT[:], start=True, stop=True)
                at = apool.tile([128, NLOC], bf, tag=f"a0_{m}",
                                name=f"a0_{b}_{m}")
                nc.scalar.activation(at[:], ps[:], relu, bias=b0t[:, m:m + 1])
                a0.append(at)
            # layer 1: [1024] -> [1024]
            a1 = []
            for m in range(KT):
                ps = pspool.tile([128, NLOC], f32, tag="ps")
                for k in range(KT):
                    lo = k * H + m * 128
                    nc.tensor.matmul(ps[:], mw1[:, lo:lo + 128], a0[k][:],
                                     start=(k == 0), stop=(k == KT - 1))
                at = apool.tile([128, NLOC], bf, tag=f"a1_{m}",
                                name=f"a1_{b}_{m}")
                nc.scalar.activation(at[:], ps[:], relu, bias=b1t[:, m:m + 1])
                a1.append(at)
            # layer 2: [1024] -> [1024]
            a2 = []
            for m in range(KT):
                ps = pspool.tile([128, NLOC], f32, tag="ps")
                for k in range(KT):
                    lo = k * H + m * 128
                    nc.tensor.matmul(ps[:], mw2[:, lo:lo + 128], a1[k][:],
                                     start=(k == 0), stop=(k == KT - 1))
                at = apool.tile([128, NLOC], bf, tag=f"a2_{m}",
                                name=f"a2_{b}_{m}")
                nc.scalar.activation(at[:], ps[:], relu, bias=b2t[:, m:m + 1])
                a2.append(at)
            # layer 3: [1024] -> [64]; all 16 masks accumulate in one psum
            for k in range(KT):
                lo = k * D_OUT
                nc.tensor.matmul(ps3[:], mw3[:, lo:lo + D_OUT], a2[k][:],
                                 start=(b == 0 and k == 0),
                                 stop=(b == B - 1 and k == KT - 1))

        # ---- finalize: ensemble mean + bias, transpose, store ----
        fin = finp.tile([D_OUT, NLOC], f32, tag="fin")
        nc.scalar.activation(fin[:], ps3[:], iden, bias=b3t[:, 0:1],
                             scale=1.0 / B)
        for t in range(NLOC // 128):
            pst = pstp.tile([128, D_OUT], f32, tag="pst")
            nc.tensor.transpose(pst[:], fin[:, t * 128:(t + 1) * 128],
                                ident[:D_OUT, :D_OUT])
            ot = outtp.tile([128, D_OUT], f32, tag="ot")
            nc.vector.tensor_copy(ot[:], pst[:])
            nc.sync.dma_start(out_d[t * 128:(t + 1) * 128, :], ot[:])

    nc.compile()
    return nc


def _prep_shared(W0, W1, W2, W3, b0, b1, b2, b3,
                 mask0, mask1, mask2, mask3):
    wt0 = np.ascontiguousarray(W0.T).astype(BF16)

    def kfold(wT, out_w):
        # [1024, out] -> [8, 128, out] -> [128, 8*out]
        return np.ascontiguousarray(
            wT.reshape(KT, 128, out_w).transpose(1, 0, 2).reshape(128, KT * out_w)
        ).astype(BF16)

    def mkfold(m, out_w):
        # [B, 1024, out] -> [B, 8, 128, out] -> [B, 128, 8*out]
        return np.ascontiguousarray(
            m.reshape(B, KT, 128, out_w).transpose(0, 2, 1, 3)
            .reshape(B, 128, KT * out_w)).astype(BF16)

    return dict(
        wt0=wt0, wt1=kfold(W1.T, H), wt2=kfold(W2.T, H),
        wt3=kfold(W3.T, D_OUT),
        m0=mask0.astype(BF16),
        m1=mkfold(mask1, H), m2=mkfold(mask2, H), m3=mkfold(mask3, D_OUT),
        b0r=np.ascontiguousarray(b0.reshape(KT, 128).T).astype(np.float32),
        b1r=np.ascontiguousarray(b1.reshape(KT, 128).T).astype(np.float32),
        b2r=np.ascontiguousarray(b2.reshape(KT, 128).T).astype(np.float32),
        b3r=np.ascontiguousarray(b3.reshape(D_OUT, 1)).astype(np.float32),
    )


def kernel(xy, W0, b0, W1, b1, W2, b2, W3, b3,
           mask0, mask1, mask2, mask3):
    global LAST_RESULT
    xy = np.asarray(xy, np.float32)
    args = [np.asarray(a, np.float32) for a in
            (W0, W1, W2, W3, b0, b1, b2, b3)]
    masks = [np.asarray(m, np.float32) for m in (mask0, mask1, mask2, mask3)]

    if "nc" not in _CACHE:
        _CACHE["nc"] = _build_graph()
    nc = _CACHE["nc"]

    shared = _prep_shared(*args, *masks)
    xyT = np.ascontiguousarray(xy.T).astype(BF16)   # [96, 4096]
    in_maps = []
    for core in range(NCORES):
        im = dict(shared)
        im["xyT"] = np.ascontiguousarray(
            xyT[:, core * NLOC:(core + 1) * NLOC])
        in_maps.append(im)

    if TRACE:
        _ensure_ntff_hook()
    res = run_bass_kernel_spmd(
        nc, in_maps, core_ids=list(range(NCORES)),
        trace=TRACE)
    LAST_RESULT = res
    return np.concatenate(
        [np.asarray(res.results[i]["out"], np.float32)
         for i in range(NCORES)], axis=0)
